# revision 1
# baseline (speedup 1.0000x reference)
"""TRN2 Bass kernel for nn_Model_48928267436601 (moe_routing).

Math: per sample b (8192 total, data-parallel over 8 cores, 1024 each):
  pg    = normalized periodogram of zero-padded FFT(x - mean)   [2048]
  gate  = pg @ Wg.T + bg ; top-2 softmax over 8 experts
  out   = w0*mean + w1*last + sum_j w_{2+j} * (sd * (xn @ Wr[j].T + br[j]) + mu)

Implementation notes:
  - The zero-padded real FFT periodogram == two matmuls against
    host-precomputed cos/sin DFT matrices [512, 2048], done as a 3-pass
    fp16 hi/lo compensated product (~3e-7 rel err at 1 cycle/row; fp32
    matmul is 4 cycles/row on TRN2).
  - pg is normalized by its own sum, so it is scale-invariant in x, and the
    RevIN scale cancels through the RLinear denorm ((x0/sd)@Wr*sd == x0@Wr),
    so only mean-removal is applied; one transposed fp16 hi/lo copy of x0
    serves both the gating DFT and the RLinear experts.
  - gate bias is folded into the gating matmul (Wg + bg works because
    sum_k pg = 1); an extra all-ones column computes the normalizer s.
  - top-2 + softmax computed densely with max8 + iota/mask compare tricks.
"""

import os
import sys

for _p in ("/opt/trn_rl_repo",):
    if _p not in sys.path and os.path.isdir(_p):
        sys.path.insert(0, _p)

import numpy as np

import concourse.bass as bass
import concourse.tile as tile
from concourse import bacc, mybir
from concourse.bass_utils import run_bass_kernel_spmd

AF = mybir.ActivationFunctionType
OP = mybir.AluOpType
FP32 = mybir.dt.float32
FP16 = mybir.dt.float16

N_CORES = 8
B, L, P = 8192, 512, 96
FFT = 4096
KF = FFT // 2          # 2048 frequencies
ER = 6
E = 2 + ER
EPS = 1e-5
B_LOC = B // N_CORES   # 1024 samples per core
NB = B_LOC // 128      # 8 row-chunks of 128 samples
NL = L // 128          # 4 chunks of the time/contraction dim
NK = KF // 128         # 16 chunks of the frequency dim
JP = ER * P            # 576 = flattened (expert, pred) dim
BIG = 1024.0


def _build_bass(include_br=True):
    nc = bacc.Bacc("TRN2", target_bir_lowering=False)

    xw = nc.declare_dram_parameter("xw", [B_LOC, L], FP32, isOutput=False)
    cosh = nc.declare_dram_parameter("cosh", [L, KF], FP16, isOutput=False)
    cosl = nc.declare_dram_parameter("cosl", [L, KF], FP16, isOutput=False)
    sinh = nc.declare_dram_parameter("sinh", [L, KF], FP16, isOutput=False)
    sinl = nc.declare_dram_parameter("sinl", [L, KF], FP16, isOutput=False)
    wga = nc.declare_dram_parameter("wga", [KF, E + 1], FP32, isOutput=False)
    wrt = nc.declare_dram_parameter("wrt", [L, JP], FP16, isOutput=False)
    brr = nc.declare_dram_parameter("brr", [1, JP], FP16, isOutput=False)
    iot = nc.declare_dram_parameter("iot", [128, 4 * E], FP32, isOutput=False)
    idn = nc.declare_dram_parameter("idn", [128, 128], FP32, isOutput=False)
    y = nc.declare_dram_parameter("y", [B_LOC, P], FP32, isOutput=True)

    with tile.TileContext(nc) as tc:
        _emit(nc, tc, xw, cosh, cosl, sinh, sinl, wga, wrt, brr, iot, idn, y,
              include_br)
    nc.compile()
    return nc


def _emit(nc, tc, xw, cosh, cosl, sinh, sinl, wga, wrt, brr, iot, idn, y, include_br):
    from contextlib import ExitStack

    ctx = ExitStack()
    with ctx:
        const = ctx.enter_context(tc.tile_pool(name="const", bufs=1))
        sml = ctx.enter_context(tc.tile_pool(name="sml", bufs=6))
        xnp = ctx.enter_context(tc.tile_pool(name="xnp", bufs=4))
        sqp = ctx.enter_context(tc.tile_pool(name="sqp", bufs=4))
        outp = ctx.enter_context(tc.tile_pool(name="outp", bufs=4))
        ps_dft = ctx.enter_context(tc.tile_pool(name="ps_dft", bufs=2, space="PSUM"))
        ps_tpg = ctx.enter_context(tc.tile_pool(name="ps_tpg", bufs=2, space="PSUM"))
        ps_rl = ctx.enter_context(tc.tile_pool(name="ps_rl", bufs=2, space="PSUM"))

        # ---- constants / inputs to SBUF (issue order ~= need order) ----
        xw_sb = const.tile([128, NB, L], FP32)
        ident = const.tile([128, 128], FP32)
        nc.sync.dma_start(
            out=xw_sb[:, 0:1, :],
            in_=xw[:, :][0:128, :].rearrange("(t p) l -> p t l", p=128),
        )
        nc.sync.dma_start(out=ident, in_=idn[:, :])
        cosh_sb = const.tile([128, NL, KF], FP16)
        cosl_sb = const.tile([128, NL, KF], FP16)
        sinh_sb = const.tile([128, NL, KF], FP16)
        sinl_sb = const.tile([128, NL, KF], FP16)
        trigs = ((cosh_sb, cosh), (cosl_sb, cosl), (sinh_sb, sinh), (sinl_sb, sinl))
        def trig_slice(ks, ke):
            for sb_t, dr in trigs:
                nc.sync.dma_start(
                    out=sb_t[:, :, ks:ke],
                    in_=dr[:, :][:, ks:ke].rearrange("(t p) k -> p t k", p=128),
                )
        def xw_dma(lo, hi):
            nc.sync.dma_start(
                out=xw_sb[:, lo:hi, :],
                in_=xw[:, :][lo * 128:hi * 128, :].rearrange("(t p) l -> p t l", p=128),
            )
        xw_dma(1, 4)
        trig_slice(0, 512)
        xw_dma(4, 8)
        for q in range(1, 4):
            trig_slice(q * 512, (q + 1) * 512)
        wga_sb = const.tile([128, NK, E + 1], FP32)
        nc.sync.dma_start(out=wga_sb, in_=wga[:, :].rearrange("(t p) e -> p t e", p=128))
        iota_sb = const.tile([128, 4, E], FP32)
        nc.sync.dma_start(out=iota_sb, in_=iot[:, :].rearrange("p (g e) -> p g e", g=4))
        brr_sb = const.tile([1, JP], FP16)
        nc.sync.dma_start(out=brr_sb, in_=brr[:, :])
        wrt_sb = const.tile([128, NL, JP], FP16)
        nc.sync.dma_start(out=wrt_sb, in_=wrt[:, :].rearrange("(t p) j -> p t j", p=128))
        ones_sb = const.tile([1, 128], FP16)
        nc.vector.memset(ones_sb, 1.0)
        eps_sb = const.tile([128, 1], FP32)
        nc.vector.memset(eps_sb, EPS)

        xh_sb = const.tile([128, NL, B_LOC], FP16)    # fp16 hi of xn^T [l, b]
        xl_sb = const.tile([128, NL, B_LOC], FP16)    # fp16 lo of xn^T [l, b]
        I_sb = const.tile([128, NK, 512], FP32)       # I^T   [k, b] per 512-col chunk
        stats = const.tile([128, NB, 4], FP32)        # mu, sd, rstd per chunk
        w_all = const.tile([128, NB, E], FP32)        # dense top-2 weights

        # ---- stats + xn + transpose, per 128-sample chunk ----
        for t in range(NB):
            x_t = xw_sb[:, t, :]
            bn6 = sml.tile([128, 6], FP32, tag="bn6")
            nc.vector.bn_stats(out=bn6, in_=x_t)
            mv = sml.tile([128, 2], FP32, tag="mv")
            nc.vector.bn_aggr(out=mv, in_=bn6)
            nc.vector.tensor_copy(stats[:, t, 0:1], mv[:, 0:1])          # mu
            xn_t = xnp.tile([128, L], FP32, tag="xn")
            if include_br:
                # exact RevIN path: xn = (x - mu) * rsqrt(var + eps)
                nc.scalar.activation(stats[:, t, 1:2], mv[:, 1:2], AF.Sqrt,
                                     bias=eps_sb)                     # sd
                nc.vector.reciprocal(stats[:, t, 2:3], stats[:, t, 1:2])
                nc.vector.tensor_scalar(
                    out=xn_t, in0=x_t,
                    scalar1=stats[:, t, 0:1], scalar2=stats[:, t, 2:3],
                    op0=OP.subtract, op1=OP.mult,
                )
            else:
                # br == 0: RevIN scale cancels ((x0/sd)@Wr*sd == x0@Wr) and the
                # normalized periodogram is scale-invariant -> mean-removal only
                nc.vector.tensor_scalar(
                    out=xn_t, in0=x_t, scalar1=stats[:, t, 0:1], scalar2=None,
                    op0=OP.subtract,
                )
            tp4 = ps_tpg.tile([128, NL, 128], FP32, tag="tpg")
            for i in range(NL):
                nc.tensor.transpose(tp4[:, i, :], xn_t[:, i * 128:(i + 1) * 128], ident)
            xh_v = xh_sb[:, :, t * 128:(t + 1) * 128]
            nc.scalar.copy(out=xh_v, in_=tp4)
            nc.vector.tensor_tensor(
                out=xl_sb[:, :, t * 128:(t + 1) * 128], in0=tp4, in1=xh_v,
                op=OP.subtract,
            )

        # ---- RLinear experts + combine for one 128-sample chunk ----
        def rl_combine(t):
            rps0 = ps_rl.tile([128, 512], FP32, tag="rl")
            rps1 = ps_rl.tile([128, 512], FP32, tag="rl")
            rps = (rps0, rps1)
            for li in range(NL):
                for h in range(2):
                    nc.tensor.matmul(
                        rps[h][:, 0:288],
                        lhsT=xh_sb[:, li, t * 128:(t + 1) * 128],
                        rhs=wrt_sb[:, li, h * 288:(h + 1) * 288],
                        start=(li == 0),
                        stop=(not include_br and li == NL - 1),
                    )
            if include_br:
                for h in range(2):  # + br via ones-row (K=1) matmul
                    nc.tensor.matmul(
                        rps[h][:, 0:288],
                        lhsT=ones_sb,
                        rhs=brr_sb[:, h * 288:(h + 1) * 288],
                        start=False,
                        stop=True,
                    )
            rl_sb = outp.tile([128, 2, 288], FP32, tag="rlsb")
            for h in range(2):
                nc.scalar.copy(out=rl_sb[:, h, :], in_=rps[h][:, 0:288])
            w_t = w_all[:, t, :]
            acc = outp.tile([128, P], FP32, tag="acc")
            nc.vector.tensor_scalar_mul(
                out=acc, in0=rl_sb[:, 0, 0:P], scalar1=w_t[:, 2:3]
            )
            for j in range(1, ER):
                h, q = j // 3, j % 3
                nc.vector.scalar_tensor_tensor(
                    out=acc, in0=rl_sb[:, h, q * P:(q + 1) * P],
                    scalar=w_t[:, 2 + j:3 + j], in1=acc,
                    op0=OP.mult, op1=OP.add,
                )
            wrsum = sml.tile([128, 1], FP32, tag="wrsum")
            nc.vector.tensor_reduce(
                out=wrsum, in_=w_t[:, 2:E], axis=mybir.AxisListType.X, op=OP.add
            )
            a1 = sml.tile([128, 1], FP32, tag="a1")
            nc.vector.tensor_mul(a1, w_t[:, 0:1], stats[:, t, 0:1])
            a2 = sml.tile([128, 1], FP32, tag="a2")
            nc.vector.scalar_tensor_tensor(
                out=a2, in0=xw_sb[:, t, L - 1:L], scalar=w_t[:, 1:2], in1=a1,
                op0=OP.mult, op1=OP.add,
            )
            a3 = sml.tile([128, 1], FP32, tag="a3")
            nc.vector.scalar_tensor_tensor(
                out=a3, in0=stats[:, t, 0:1], scalar=wrsum, in1=a2,
                op0=OP.mult, op1=OP.add,
            )
            y_t = outp.tile([128, P], FP32, tag="y")
            if include_br:
                nc.vector.tensor_scalar(
                    out=y_t, in0=acc, scalar1=stats[:, t, 1:2], scalar2=a3,
                    op0=OP.mult, op1=OP.add,
                )
            else:
                nc.vector.tensor_scalar_add(out=y_t, in0=acc, scalar1=a3)
            nc.sync.dma_start(out=y[:, :][t * 128:(t + 1) * 128, :], in_=y_t)



        # ---- per 512-sample chunk: DFT -> I^T -> gate -> top2 weights ----
        for c in range(B_LOC // 512):
            bsl = slice(c * 512, (c + 1) * 512)
            for kc in range(NK):
                ksl = slice(kc * 128, (kc + 1) * 128)
                ps = ps_dft.tile([128, 2, 512], FP32, tag="dft")
                for h, (th, tl) in enumerate(((cosh_sb, cosl_sb), (sinh_sb, sinl_sb))):
                    passes = [(th, xh_sb), (tl, xh_sb), (th, xl_sb)]
                    for pi, (trig_sb, xs) in enumerate(passes):
                        for li in range(NL):
                            nc.tensor.matmul(
                                ps[:, h, :],
                                lhsT=trig_sb[:, li, ksl],
                                rhs=xs[:, li, bsl],
                                start=(pi == 0 and li == 0),
                                stop=(pi == 2 and li == NL - 1),
                            )
                sq = sqp.tile([128, 2, 512], FP32, tag="sq")
                nc.scalar.activation(sq, ps, AF.Square)
                nc.vector.tensor_add(I_sb[:, kc, :], sq[:, 0, :], sq[:, 1, :])

            gps = ps_tpg.tile([128, 4, E + 1], FP32, tag="tpg")
            for s in range(4):
                for kc in range(NK):
                    nc.tensor.matmul(
                        gps[:, s, :],
                        lhsT=I_sb[:, kc, s * 128:(s + 1) * 128],
                        rhs=wga_sb[:, kc, :],
                        start=(kc == 0),
                        stop=(kc == NK - 1),
                    )
            # ---- batched dense top-2 softmax weights ([128, 4, 8]) ----
            G3 = [128, 4, E]
            sc = sml.tile([128, 4], FP32, tag="sc")
            nc.vector.tensor_scalar_add(out=sc, in0=gps[:, :, E], scalar1=1e-38)
            rs = sml.tile([128, 4], FP32, tag="rs")
            nc.vector.reciprocal(rs, sc)
            gg = sml.tile(G3, FP32, tag="gg")
            nc.vector.tensor_tensor(out=gg, in0=gps[:, :, 0:E], in1=rs.to_broadcast(G3), op=OP.mult)
            m1 = sml.tile([128, 4], FP32, tag="m1")
            nc.vector.tensor_reduce(out=m1, in_=gg, axis=mybir.AxisListType.X, op=OP.max)
            eq1 = sml.tile(G3, FP32, tag="eq1")
            nc.vector.tensor_tensor(out=eq1, in0=gg, in1=m1.to_broadcast(G3), op=OP.is_equal)
            t1 = sml.tile(G3, FP32, tag="t1")
            nc.vector.tensor_scalar_mul(out=t1, in0=eq1, scalar1=-BIG)
            t8 = sml.tile(G3, FP32, tag="t8")
            nc.vector.tensor_add(t8, t1, iota_sb)
            idx1 = sml.tile([128, 4], FP32, tag="idx1")
            nc.vector.tensor_reduce(out=idx1, in_=t8, axis=mybir.AxisListType.X, op=OP.min)
            nc.vector.tensor_scalar_add(out=idx1, in0=idx1, scalar1=BIG)
            eqf = sml.tile(G3, FP32, tag="eqf")
            nc.vector.tensor_tensor(out=eqf, in0=iota_sb, in1=idx1.to_broadcast(G3), op=OP.is_equal)
            t2 = sml.tile(G3, FP32, tag="t2")
            nc.vector.tensor_scalar_mul(out=t2, in0=eqf, scalar1=-1e30)
            gm = sml.tile(G3, FP32, tag="gm")
            nc.vector.tensor_add(gm, t2, gg)
            m2 = sml.tile([128, 4], FP32, tag="m2")
            nc.vector.tensor_reduce(out=m2, in_=gm, axis=mybir.AxisListType.X, op=OP.max)
            eq2 = sml.tile(G3, FP32, tag="eq2")
            nc.vector.tensor_tensor(out=eq2, in0=gm, in1=m2.to_broadcast(G3), op=OP.is_equal)
            t3 = sml.tile(G3, FP32, tag="t3")
            nc.vector.tensor_scalar_mul(out=t3, in0=eq2, scalar1=-BIG)
            t4 = sml.tile(G3, FP32, tag="t4")
            nc.vector.tensor_add(t4, t3, iota_sb)
            t5 = sml.tile(G3, FP32, tag="t5")
            nc.vector.tensor_scalar_mul(out=t5, in0=eqf, scalar1=2.0 * BIG)
            t6 = sml.tile(G3, FP32, tag="t6")
            nc.vector.tensor_add(t6, t4, t5)
            idx2 = sml.tile([128, 4], FP32, tag="idx2")
            nc.vector.tensor_reduce(out=idx2, in_=t6, axis=mybir.AxisListType.X, op=OP.min)
            nc.vector.tensor_scalar_add(out=idx2, in0=idx2, scalar1=BIG)
            eqf2 = sml.tile(G3, FP32, tag="eqf2")
            nc.vector.tensor_tensor(out=eqf2, in0=iota_sb, in1=idx2.to_broadcast(G3), op=OP.is_equal)
            sel = sml.tile(G3, FP32, tag="sel")
            nc.vector.tensor_add(sel, eqf, eqf2)
            ex = sml.tile(G3, FP32, tag="ex")
            nc.scalar.activation(ex, gg, AF.Exp)   # |g| << 1, no max-subtraction needed
            wraw = sml.tile(G3, FP32, tag="wraw")
            nc.vector.tensor_mul(wraw, ex, sel)
            z = sml.tile([128, 4], FP32, tag="z")
            nc.vector.tensor_reduce(out=z, in_=wraw, axis=mybir.AxisListType.X, op=OP.add)
            rz = sml.tile([128, 4], FP32, tag="rz")
            nc.vector.reciprocal(rz, z)
            nc.vector.tensor_tensor(
                out=w_all[:, 4 * c:4 * c + 4, :], in0=wraw,
                in1=rz.to_broadcast(G3), op=OP.mult,
            )

        for t in range(NB):
            rl_combine(t)


_CACHE = {}


def _get_nc(include_br=True):
    key = ("nc", include_br)
    if key not in _CACHE:
        _CACHE[key] = _build_bass(include_br)
    return _CACHE[key]


def _host_constants(Wg, bg, Wr, br):
    ll = np.arange(L, dtype=np.float64)
    kk = np.arange(KF, dtype=np.float64)
    ang = 2.0 * np.pi * np.outer(ll, kk) / FFT
    cosM = np.cos(ang)
    sinM = np.sin(ang)
    cosH = cosM.astype(np.float16)
    cosL = (cosM - cosH.astype(np.float64)).astype(np.float16)
    sinH = sinM.astype(np.float16)
    sinL = (sinM - sinH.astype(np.float64)).astype(np.float16)
    wga = np.concatenate(
        [
            (Wg.astype(np.float32) + bg.astype(np.float32)[:, None]).T,
            np.ones((KF, 1), np.float32),
        ],
        axis=1,
    )
    wrt = np.ascontiguousarray(
        Wr.astype(np.float16).transpose(2, 0, 1).reshape(L, JP)
    )
    brr = np.ascontiguousarray(br.astype(np.float16).reshape(1, JP))
    iot = np.tile(np.arange(E, dtype=np.float32), (128, 4))
    idn = np.eye(128, dtype=np.float32)
    return cosH, cosL, sinH, sinL, wga, wrt, brr, iot, idn


def kernel(x, Wg, bg, Wr, br, **_unused):
    x = np.ascontiguousarray(np.asarray(x, dtype=np.float32))
    cosH, cosL, sinH, sinL, wga, wrt, brr, iot, idn = _host_constants(
        np.asarray(Wg), np.asarray(bg), np.asarray(Wr), np.asarray(br)
    )
    nc = _get_nc(include_br=bool(np.any(np.asarray(br))))
    core_ids = list(range(N_CORES))
    in_maps = []
    for i in core_ids:
        in_maps.append(
            {
                "xw": np.ascontiguousarray(x[i * B_LOC:(i + 1) * B_LOC]),
                "cosh": cosH, "cosl": cosL, "sinh": sinH, "sinl": sinL,
                "wga": wga,
                "wrt": wrt,
                "brr": brr,
                "iot": iot,
                "idn": idn,
            }
        )
    res = run_bass_kernel_spmd(nc, in_maps, core_ids)
    out = np.concatenate([res.results[i]["y"] for i in core_ids], axis=0)
    return out.astype(np.float32)


def profile_once(inputs, tmpdir=None):
    """Run once with tracing; returns exec_time_ns (or None if unavailable)."""
    x = np.ascontiguousarray(np.asarray(inputs["x"], dtype=np.float32))
    cosH, cosL, sinH, sinL, wga, wrt, brr, iot, idn = _host_constants(
        np.asarray(inputs["Wg"]), np.asarray(inputs["bg"]),
        np.asarray(inputs["Wr"]), np.asarray(inputs["br"]),
    )
    nc = _get_nc()
    core_ids = list(range(N_CORES))
    in_maps = [
        {
            "xw": np.ascontiguousarray(x[i * B_LOC:(i + 1) * B_LOC]),
            "cosh": cosH, "cosl": cosL, "sinh": sinH, "sinl": sinL, "wga": wga,
            "wrt": wrt, "brr": brr, "iot": iot, "idn": idn,
        }
        for i in core_ids
    ]
    try:
        res = run_bass_kernel_spmd(nc, in_maps, core_ids, trace=True, tmpdir=tmpdir)
        print("profile_json:", res.profile_json)
        print("mean_exec_time_ns:", res.mean_exec_time_ns,
              "max core:", res.max_exec_time_core_id)
        return res.exec_time_ns
    except Exception as exc:  # noqa: BLE001
        print("profiling failed:", exc)
        return None


if __name__ == "__main__":
    rng = np.random.default_rng(0)
    demo = {
        "x": rng.standard_normal((B, L), dtype=np.float32),
        "Wg": (rng.standard_normal((E, KF)) * 0.02).astype(np.float32),
        "bg": np.zeros((E,), np.float32),
        "Wr": (rng.standard_normal((ER, P, L)) * 0.02).astype(np.float32),
        "br": np.zeros((ER, P), np.float32),
    }
    print(kernel(**demo).shape)



# revision 7
# speedup vs baseline: 2.1431x; 2.1431x over previous
"""TRN2 Bass kernel for nn_Model_48928267436601 (moe_routing).

Math: per sample b (8192 total, data-parallel over 8 cores, 1024 each):
  pg    = normalized periodogram of zero-padded FFT(x - mean)   [2048]
  gate  = pg @ Wg.T + bg ; top-2 softmax over 8 experts
  out   = w0*mean + w1*last + sum_j w_{2+j} * (sd * (xn @ Wr[j].T + br[j]) + mu)

Implementation notes:
  - The zero-padded real FFT periodogram == two matmuls against
    host-precomputed cos/sin DFT matrices [512, 2048]. These run as a SINGLE
    float32r pass: the PE reads f32r at 13 mantissa bits (FP22 truncation) at
    1 cycle/row -- 3x cheaper than the fp16 hi/lo compensated scheme and
    ~16x more accurate than a single fp16 pass (the top-2 gate margins need
    ~1e-6 gate accuracy; measured gate err ~2e-7, zero routing flips).
    Host constants are pre-ROUNDED to 13 mantissa bits so the hardware
    truncation is exact on them.
  - pg is normalized by its own sum, so it is scale-invariant in x, and the
    RevIN scale cancels through the RLinear denorm ((x0/sd)@Wr*sd == x0@Wr),
    so only mean-removal is applied (br == 0 path).
  - gate bias is folded into the gating matmul (Wg + bg works because
    sum_k pg = 1); an extra all-ones column computes the normalizer s.
  - DFT runs kc-quarter-major so the trig DMA (8MB fp32) streams ahead of
    the PE; the gate matmul accumulates into persistent PSUM tiles across
    quarters and is emitted 2 DFT tiles into the following quarter so the
    PE never waits on the Act/Pool square+add pipeline.
  - top-2 + softmax computed densely with max8 + iota/mask compare tricks.
  - Engine placement: squares on Act, I-adds + means on Pool (otherwise
    idle), transpose copies/xn/top2/combine on DVE; the expert combine runs
    in fp16 (DVE 4x mode) -- output tolerance is 2e-2, fp16 is plenty there.
"""

import os
import sys

for _p in ("/opt/trn_rl_repo",):
    if _p not in sys.path and os.path.isdir(_p):
        sys.path.insert(0, _p)

import numpy as np

import concourse.bass as bass
import concourse.tile as tile
from concourse import bacc, mybir
from concourse.bass_utils import run_bass_kernel_spmd

AF = mybir.ActivationFunctionType
OP = mybir.AluOpType
FP32 = mybir.dt.float32
F32R = mybir.dt.float32r
FP16 = mybir.dt.float16

N_CORES = 8
B, L, P = 8192, 512, 96
FFT = 4096
KF = 2048          # 2048 frequencies
ER = 6
E = 2 + ER
EPS = 1e-5
B_LOC = B // N_CORES   # 1024 samples per core
NB = B_LOC // 128      # 8 row-chunks of 128 samples
NL = L // 128          # 4 chunks of the time/contraction dim
NK = KF // 128         # 16 chunks of the frequency dim
NC_B = B_LOC // 512    # 2 chunks of 512 samples (DFT moving-operand width)
NQ = 4                 # trig quarters (4 kc each)
JP = ER * P            # 576 = flattened (expert, pred) dim
BIG = 1024.0


def _build_bass(include_br=True):
    nc = bacc.Bacc("TRN2", target_bir_lowering=False)

    xw = nc.declare_dram_parameter("xw", [B_LOC, L], FP32, isOutput=False)
    cosq = nc.declare_dram_parameter("cosq", [L, KF], F32R, isOutput=False)
    sinq = nc.declare_dram_parameter("sinq", [L, KF], F32R, isOutput=False)
    wga = nc.declare_dram_parameter("wga", [KF, E + 2], F32R, isOutput=False)
    wrt = nc.declare_dram_parameter("wrt", [L, JP], F32R, isOutput=False)
    brr = nc.declare_dram_parameter("brr", [1, JP], F32R, isOutput=False)
    iot = nc.declare_dram_parameter("iot", [128, 4 * E], FP32, isOutput=False)
    one = nc.declare_dram_parameter("one", [1, 128], F32R, isOutput=False)
    idn = nc.declare_dram_parameter("idn", [128, 128], F32R, isOutput=False)
    y = nc.declare_dram_parameter("y", [B_LOC, P], FP32, isOutput=True)

    with tile.TileContext(nc) as tc:
        _emit(nc, tc, xw, cosq, sinq, wga, wrt, brr, iot, idn, one, y, include_br)
    nc.compile()
    return nc


def _emit(nc, tc, xw, cosq, sinq, wga, wrt, brr, iot, idn, one, y, include_br):
    from contextlib import ExitStack

    ctx = ExitStack()
    with ctx:
        const = ctx.enter_context(tc.tile_pool(name="const", bufs=1))
        sml = ctx.enter_context(tc.tile_pool(name="sml", bufs=6))
        xnp = ctx.enter_context(tc.tile_pool(name="xnp", bufs=4))
        sqp = ctx.enter_context(tc.tile_pool(name="sqp", bufs=4))
        iqp = ctx.enter_context(tc.tile_pool(name="iqp", bufs=2))
        outp = ctx.enter_context(tc.tile_pool(name="outp", bufs=4))
        ps_dft = ctx.enter_context(tc.tile_pool(name="ps_dft", bufs=2, space="PSUM"))
        ps_tpg = ctx.enter_context(tc.tile_pool(name="ps_tpg", bufs=2, space="PSUM"))
        ps_rl = ctx.enter_context(tc.tile_pool(name="ps_rl", bufs=2, space="PSUM"))

        # ---- constants / inputs to SBUF (issue order ~= need order) ----
        xw_sb = const.tile([128, NB, L], FP32)
        ident = const.tile([128, 128], F32R)
        iota_sb = const.tile([128, 4, E], FP32)

        def xw_dma(lo, hi):
            nc.sync.dma_start(
                out=xw_sb[:, lo:hi, :],
                in_=xw[:, :][lo * 128:hi * 128, :].rearrange("(t p) l -> p t l", p=128),
            )

        xw_dma(0, 4)
        nc.sync.dma_start(out=ident, in_=idn[:, :])
        nc.sync.dma_start(out=iota_sb, in_=iot[:, :].rearrange("p (g e) -> p g e", g=4))

        cos_sb = const.tile([128, NL, KF], F32R)
        sin_sb = const.tile([128, NL, KF], F32R)

        def trig_slice(q):
            ks, ke = q * 512, (q + 1) * 512
            for sb_t, dr in ((cos_sb, cosq), (sin_sb, sinq)):
                nc.sync.dma_start(
                    out=sb_t[:, :, ks:ke],
                    in_=dr[:, :][:, ks:ke].rearrange("(t p) k -> p t k", p=128),
                )

        trig_slice(0)
        xw_dma(4, 8)
        wga_sb = const.tile([128, NK, E + 2], F32R)
        nc.sync.dma_start(out=wga_sb, in_=wga[:, :].rearrange("(t p) e -> p t e", p=128))
        trig_slice(1)
        wrt_sb = const.tile([128, NL, JP], F32R)
        nc.sync.dma_start(out=wrt_sb, in_=wrt[:, :].rearrange("(t p) j -> p t j", p=128))
        trig_slice(2)
        trig_slice(3)
        brr_sb = const.tile([1, JP], F32R)
        nc.sync.dma_start(out=brr_sb, in_=brr[:, :])
        ones_sb = const.tile([1, 128], F32R)
        nc.sync.dma_start(out=ones_sb, in_=one[:, :])
        eps_sb = const.tile([128, 1], FP32)
        nc.vector.memset(eps_sb, EPS)

        x0T = const.tile([128, NL, B_LOC], F32R)      # (x - mu)^T  [l, b]
        stats = const.tile([128, NB, 4], FP32)        # mu (and sd, rstd if br)
        w_all = const.tile([128, NB, E], FP32)        # dense top-2 weights

        # ---- stats + xn + transpose, per 128-sample chunk ----
        for t in range(NB):
            x_t = xw_sb[:, t, :]
            if include_br:
                bn6 = sml.tile([128, 6], FP32, tag="bn6")
                nc.vector.bn_stats(out=bn6, in_=x_t)
                mv = sml.tile([128, 2], FP32, tag="mv")
                nc.vector.bn_aggr(out=mv, in_=bn6)
                nc.vector.tensor_copy(stats[:, t, 0:1], mv[:, 0:1])      # mu
                nc.scalar.activation(stats[:, t, 1:2], mv[:, 1:2], AF.Sqrt,
                                     bias=eps_sb)                        # sd
                nc.vector.reciprocal(stats[:, t, 2:3], stats[:, t, 1:2])
                xn_t = xnp.tile([128, L], F32R, tag="xn")
                nc.vector.tensor_scalar(
                    out=xn_t, in0=x_t,
                    scalar1=stats[:, t, 0:1], scalar2=stats[:, t, 2:3],
                    op0=OP.subtract, op1=OP.mult,
                )
            else:
                # scale cancels (br==0) so mean-removal only; sum on DVE,
                # scale + subtract on the otherwise-idle Pool engine
                nc.vector.tensor_reduce(
                    out=stats[:, t, 1:2], in_=x_t, axis=mybir.AxisListType.X,
                    op=OP.add,
                )
                nc.gpsimd.tensor_scalar_mul(
                    out=stats[:, t, 0:1], in0=stats[:, t, 1:2], scalar1=1.0 / L
                )
                xn_t = xnp.tile([128, L], F32R, tag="xn")
                nc.gpsimd.tensor_scalar(
                    out=xn_t, in0=x_t, scalar1=stats[:, t, 0:1], scalar2=None,
                    op0=OP.subtract,
                )
            tp4 = ps_tpg.tile([128, NL, 128], F32R, tag="tpg")
            for i in range(NL):
                nc.tensor.transpose(tp4[:, i, :], xn_t[:, i * 128:(i + 1) * 128], ident)
            nc.vector.tensor_copy(x0T[:, :, t * 128:(t + 1) * 128], tp4)

        # persistent gate PSUM accumulators (one per 512-sample chunk);
        # allocated after the transposes so ps_tpg's two bufs are free.
        gps = []
        for _c in range(NC_B):
            gps_c = ps_tpg.tile([128, 4, E + 2], FP32, tag="tpg")
            gps.append(gps_c)

        # ---- DFT quarter: 4 kc x 2 chunks -> I_q; emits matmuls only ----
        def dft_quarter(q, iq):
            for c in range(NC_B):
                bsl = slice(c * 512, (c + 1) * 512)
                for kcq in range(4):
                    kc = 4 * q + kcq
                    ksl = slice(kc * 128, (kc + 1) * 128)
                    ps = ps_dft.tile([128, 2, 512], FP32, tag="dft")
                    for h, trig_sb in enumerate((cos_sb, sin_sb)):
                        for li in range(NL):
                            nc.tensor.matmul(
                                ps[:, h, :],
                                lhsT=trig_sb[:, li, ksl],
                                rhs=x0T[:, li, bsl],
                                start=(li == 0),
                                stop=(li == NL - 1),
                            )
                    sq = sqp.tile([128, 2, 512], FP32, tag="sq")
                    nc.scalar.activation(sq, ps, AF.Square)
                    nc.gpsimd.tensor_tensor(
                        out=iq[:, c, kcq, :], in0=sq[:, 0, :], in1=sq[:, 1, :],
                        op=OP.add,
                    )

        # ---- gate matmuls for one finished quarter (accumulate into gps) ----
        def gate_quarter(q, iq, c):
            for kcq in range(4):
                kc = 4 * q + kcq
                for s in range(4):
                    # start only on the bank's very first write: start=True
                    # marks the whole 2KB zero region, so later s-groups'
                    # first writes land on pending-zero bytes (= overwrite)
                    # and re-issuing start would wipe earlier s results.
                    nc.tensor.matmul(
                        gps[c][:, s, :],
                        lhsT=iq[:, c, kcq, s * 128:(s + 1) * 128],
                        rhs=wga_sb[:, kc, :],
                        start=(q == 0 and kcq == 0 and s == 0),
                        stop=(q == NQ - 1 and kcq == 3),
                        skip_group_check=True,
                    )

        # ---- dense top-2 softmax weights for one 512-chunk ([128, 4, 8]) ----
        def top2(c):
            G3 = [128, 4, E]
            g = gps[c]
            sc = sml.tile([128, 4], FP32, tag="sc")
            nc.vector.tensor_scalar_add(out=sc, in0=g[:, :, E], scalar1=1e-38)
            rs = sml.tile([128, 4], FP32, tag="rs")
            nc.vector.reciprocal(rs, sc)
            gg = sml.tile(G3, FP32, tag="gg")
            nc.vector.tensor_tensor(out=gg, in0=g[:, :, 0:E], in1=rs.to_broadcast(G3), op=OP.mult)
            m1 = sml.tile([128, 4], FP32, tag="m1")
            nc.vector.tensor_reduce(out=m1, in_=gg, axis=mybir.AxisListType.X, op=OP.max)
            eq1 = sml.tile(G3, FP32, tag="eq1")
            nc.vector.tensor_tensor(out=eq1, in0=gg, in1=m1.to_broadcast(G3), op=OP.is_equal)
            t1 = sml.tile(G3, FP32, tag="t1")
            nc.vector.tensor_scalar_mul(out=t1, in0=eq1, scalar1=-BIG)
            t8 = sml.tile(G3, FP32, tag="t8")
            nc.vector.tensor_add(t8, t1, iota_sb)
            idx1 = sml.tile([128, 4], FP32, tag="idx1")
            nc.vector.tensor_reduce(out=idx1, in_=t8, axis=mybir.AxisListType.X, op=OP.min)
            nc.vector.tensor_scalar_add(out=idx1, in0=idx1, scalar1=BIG)
            eqf = sml.tile(G3, FP32, tag="eqf")
            nc.vector.tensor_tensor(out=eqf, in0=iota_sb, in1=idx1.to_broadcast(G3), op=OP.is_equal)
            t2 = sml.tile(G3, FP32, tag="t2")
            nc.vector.tensor_scalar_mul(out=t2, in0=eqf, scalar1=-1e30)
            gm = sml.tile(G3, FP32, tag="gm")
            nc.vector.tensor_add(gm, t2, gg)
            m2 = sml.tile([128, 4], FP32, tag="m2")
            nc.vector.tensor_reduce(out=m2, in_=gm, axis=mybir.AxisListType.X, op=OP.max)
            eq2 = sml.tile(G3, FP32, tag="eq2")
            nc.vector.tensor_tensor(out=eq2, in0=gm, in1=m2.to_broadcast(G3), op=OP.is_equal)
            t3 = sml.tile(G3, FP32, tag="t3")
            nc.vector.tensor_scalar_mul(out=t3, in0=eq2, scalar1=-BIG)
            t4 = sml.tile(G3, FP32, tag="t4")
            nc.vector.tensor_add(t4, t3, iota_sb)
            t5 = sml.tile(G3, FP32, tag="t5")
            nc.vector.tensor_scalar_mul(out=t5, in0=eqf, scalar1=2.0 * BIG)
            t6 = sml.tile(G3, FP32, tag="t6")
            nc.vector.tensor_add(t6, t4, t5)
            idx2 = sml.tile([128, 4], FP32, tag="idx2")
            nc.vector.tensor_reduce(out=idx2, in_=t6, axis=mybir.AxisListType.X, op=OP.min)
            nc.vector.tensor_scalar_add(out=idx2, in0=idx2, scalar1=BIG)
            eqf2 = sml.tile(G3, FP32, tag="eqf2")
            nc.vector.tensor_tensor(out=eqf2, in0=iota_sb, in1=idx2.to_broadcast(G3), op=OP.is_equal)
            sel = sml.tile(G3, FP32, tag="sel")
            nc.vector.tensor_add(sel, eqf, eqf2)
            ex = sml.tile(G3, FP32, tag="ex")
            nc.scalar.activation(ex, gg, AF.Exp)   # |g| << 1, no max-subtraction
            wraw = sml.tile(G3, FP32, tag="wraw")
            nc.vector.tensor_mul(wraw, ex, sel)
            z = sml.tile([128, 4], FP32, tag="z")
            nc.vector.tensor_reduce(out=z, in_=wraw, axis=mybir.AxisListType.X, op=OP.add)
            rz = sml.tile([128, 4], FP32, tag="rz")
            nc.vector.reciprocal(rz, z)
            nc.vector.tensor_tensor(
                out=w_all[:, 4 * c:4 * c + 4, :], in0=wraw,
                in1=rz.to_broadcast(G3), op=OP.mult,
            )

        # ---- RLinear matmuls for one 128-sample chunk ----
        def rl_matmul(t):
            rps0 = ps_rl.tile([128, 512], FP32, tag="rl")
            rps1 = ps_rl.tile([128, 512], FP32, tag="rl")
            rps = (rps0, rps1)
            for li in range(NL):
                for h in range(2):
                    nc.tensor.matmul(
                        rps[h][:, 0:288],
                        lhsT=x0T[:, li, t * 128:(t + 1) * 128],
                        rhs=wrt_sb[:, li, h * 288:(h + 1) * 288],
                        start=(li == 0),
                        stop=(not include_br and li == NL - 1),
                    )
            if include_br:
                for h in range(2):  # + br via ones-row (K=1) matmul
                    nc.tensor.matmul(
                        rps[h][:, 0:288],
                        lhsT=ones_sb,
                        rhs=brr_sb[:, h * 288:(h + 1) * 288],
                        start=False,
                        stop=True,
                    )
            rl_sb = outp.tile([128, 2, 288], FP16, tag="rlsb")
            for h in range(2):
                nc.scalar.copy(out=rl_sb[:, h, :], in_=rps[h][:, 0:288])
            return rl_sb

        # ---- weighted expert combine for one 128-sample chunk (fp16) ----
        def combine(t, rl_sb):
            w_t = w_all[:, t, :]
            acc = outp.tile([128, P], FP16, tag="acc")
            nc.vector.tensor_scalar_mul(
                out=acc, in0=rl_sb[:, 0, 0:P], scalar1=w_t[:, 2:3]
            )
            for j in range(1, ER):
                h, q = j // 3, j % 3
                nc.vector.scalar_tensor_tensor(
                    out=acc, in0=rl_sb[:, h, q * P:(q + 1) * P],
                    scalar=w_t[:, 2 + j:3 + j], in1=acc,
                    op0=OP.mult, op1=OP.add,
                )
            wrsum = sml.tile([128, 1], FP32, tag="wrsum")
            nc.vector.tensor_reduce(
                out=wrsum, in_=w_t[:, 2:E], axis=mybir.AxisListType.X, op=OP.add
            )
            a1 = sml.tile([128, 1], FP32, tag="a1")
            nc.vector.tensor_mul(a1, w_t[:, 0:1], stats[:, t, 0:1])
            a2 = sml.tile([128, 1], FP32, tag="a2")
            nc.vector.scalar_tensor_tensor(
                out=a2, in0=xw_sb[:, t, L - 1:L], scalar=w_t[:, 1:2], in1=a1,
                op0=OP.mult, op1=OP.add,
            )
            a3 = sml.tile([128, 1], FP32, tag="a3")
            nc.vector.scalar_tensor_tensor(
                out=a3, in0=stats[:, t, 0:1], scalar=wrsum, in1=a2,
                op0=OP.mult, op1=OP.add,
            )
            y_t = outp.tile([128, P], FP32, tag="y")
            if include_br:
                nc.vector.tensor_scalar(
                    out=y_t, in0=acc, scalar1=stats[:, t, 1:2], scalar2=a3,
                    op0=OP.mult, op1=OP.add,
                )
            else:
                nc.vector.tensor_scalar_add(out=y_t, in0=acc, scalar1=a3)
            nc.sync.dma_start(out=y[:, :][t * 128:(t + 1) * 128, :], in_=y_t)

        # ---- schedule: quarters stream; gates trail by one quarter ----
        def iq_tile():
            iq_t = iqp.tile([128, NC_B, 4, 512], F32R, tag="iq")
            return iq_t

        iqs = [iq_tile(), iq_tile()]
        dft_quarter(0, iqs[0])
        dft_quarter(1, iqs[1])
        for c in range(NC_B):
            gate_quarter(0, iqs[0], c)
        iqs.append(iq_tile())
        dft_quarter(2, iqs[2])
        for c in range(NC_B):
            gate_quarter(1, iqs[1], c)
        iqs.append(iq_tile())
        dft_quarter(3, iqs[3])
        for c in range(NC_B):
            gate_quarter(2, iqs[2], c)
        # final-quarter gates interleaved with RL matmuls to hide the
        # square+add latency of the last tiles
        rl_tiles = {}
        rl_tiles[0] = rl_matmul(0)
        rl_tiles[1] = rl_matmul(1)
        gate_quarter(3, iqs[3], 0)
        top2(0)
        rl_tiles[2] = rl_matmul(2)
        rl_tiles[3] = rl_matmul(3)
        gate_quarter(3, iqs[3], 1)
        top2(1)
        combine(0, rl_tiles[0])
        combine(1, rl_tiles[1])
        rl_tiles[4] = rl_matmul(4)
        combine(2, rl_tiles[2])
        rl_tiles[5] = rl_matmul(5)
        combine(3, rl_tiles[3])
        rl_tiles[6] = rl_matmul(6)
        combine(4, rl_tiles[4])
        rl_tiles[7] = rl_matmul(7)
        combine(5, rl_tiles[5])
        combine(6, rl_tiles[6])
        combine(7, rl_tiles[7])


_CACHE = {}


def _get_nc(include_br=True):
    key = ("nc", include_br)
    if key not in _CACHE:
        _CACHE[key] = _build_bass(include_br)
    return _CACHE[key]


def _round13(a):
    """fp32 -> 13-mantissa-bit round-to-nearest (so the PE f32r truncation
    is exact on host-prepped constants)."""
    a32 = np.ascontiguousarray(a, dtype=np.float32)
    u = a32.view(np.uint32).astype(np.uint64)
    u = (u + np.uint64(0x200)) & np.uint64(0xFFFFFC00)
    return u.astype(np.uint32).view(np.float32)


def _host_constants(Wg, bg, Wr, br):
    ll = np.arange(L, dtype=np.float64)
    kk = np.arange(KF, dtype=np.float64)
    ang = 2.0 * np.pi * np.outer(ll, kk) / FFT
    cosM = _round13(np.cos(ang))
    sinM = _round13(np.sin(ang))
    wga = _round13(
        np.concatenate(
            [
                (Wg.astype(np.float64) + bg.astype(np.float64)[:, None]).T,
                np.ones((KF, 1), np.float64),
                np.zeros((KF, 1), np.float64),
            ],
            axis=1,
        )
    )
    wrt = _round13(
        np.ascontiguousarray(Wr.astype(np.float64).transpose(2, 0, 1).reshape(L, JP))
    )
    brr = _round13(np.ascontiguousarray(br.astype(np.float64).reshape(1, JP)))
    iot = np.tile(np.arange(E, dtype=np.float32), (128, 4))
    idn = np.eye(128, dtype=np.float32)
    one = np.ones((1, 128), dtype=np.float32)
    return cosM, sinM, wga, wrt, brr, iot, idn, one


def _in_maps(x, Wg, bg, Wr, br):
    cosM, sinM, wga, wrt, brr, iot, idn, one = _host_constants(
        np.asarray(Wg), np.asarray(bg), np.asarray(Wr), np.asarray(br)
    )
    maps = []
    for i in range(N_CORES):
        maps.append(
            {
                "xw": np.ascontiguousarray(x[i * B_LOC:(i + 1) * B_LOC]),
                "cosq": cosM, "sinq": sinM,
                "wga": wga, "wrt": wrt, "brr": brr,
                "iot": iot, "idn": idn, "one": one,
            }
        )
    return maps


def kernel(x, Wg, bg, Wr, br, **_unused):
    x = np.ascontiguousarray(np.asarray(x, dtype=np.float32))
    nc = _get_nc(include_br=bool(np.any(np.asarray(br))))
    core_ids = list(range(N_CORES))
    res = run_bass_kernel_spmd(nc, _in_maps(x, Wg, bg, Wr, br), core_ids)
    out = np.concatenate([res.results[i]["y"] for i in core_ids], axis=0)
    return out.astype(np.float32)


def profile_once(inputs, tmpdir=None):
    """Run once with tracing; returns exec_time_ns (or None if unavailable)."""
    x = np.ascontiguousarray(np.asarray(inputs["x"], dtype=np.float32))
    nc = _get_nc(include_br=bool(np.any(np.asarray(inputs["br"]))))
    core_ids = list(range(N_CORES))
    maps = _in_maps(x, inputs["Wg"], inputs["bg"], inputs["Wr"], inputs["br"])
    try:
        res = run_bass_kernel_spmd(nc, maps, core_ids, trace=True, tmpdir=tmpdir)
        print("profile_json:", res.profile_json)
        print("mean_exec_time_ns:", res.mean_exec_time_ns,
              "max core:", res.max_exec_time_core_id)
        return res.exec_time_ns
    except Exception as exc:  # noqa: BLE001
        print("profiling failed:", exc)
        return None


if __name__ == "__main__":
    rng = np.random.default_rng(0)
    demo = {
        "x": rng.standard_normal((B, L), dtype=np.float32),
        "Wg": (rng.standard_normal((E, KF)) * 0.02).astype(np.float32),
        "bg": np.zeros((E,), np.float32),
        "Wr": (rng.standard_normal((ER, P, L)) * 0.02).astype(np.float32),
        "br": np.zeros((ER, P), np.float32),
    }
    print(kernel(**demo).shape)
